# revision 1
# baseline (speedup 1.0000x reference)
"""Trainium2 Bass kernel for nn_DeformSpaceAttention (deformable 3x3 unfold +
per-channel max over taps + 1x1 conv + sigmoid).

Strategy (8 cores, data parallel over (batch, H-half)):
  - Each core handles one (sample b, 50-row half) shard: 5000 output pixels.
  - Device builds a zero-padded channels-last copy of its sample in DRAM:
    xT_pad[(h+8)*116 + (w+8), c] in bf16 (PAD=8 absorbs all out-of-bounds
    bilinear reads; clamped indices land in the zero pad, reproducing the
    reference's zero-padding semantics exactly for arbitrary offsets).
  - Per (tap, y-neighbor): SWDGE dma_gather (transpose=True) gathers, for
    every output pixel, the bf16 row-pair xT_pad[idx, 0:512] (the two
    x-neighbors of all 256 channels) straight into channel-partition layout
    [128c, (b,grp), px].
  - Bilinear corner weights are computed per-pixel on DVE, broadcast across
    the 128 channel partitions via PE transposes of free-broadcast APs, and
    the 4-corner blend + running max over the 9 taps runs on DVE in bf16.
  - 1x1 conv = PE matmul with w0; sigmoid(+bias) on ACT; store.
"""

import os
import sys
from contextlib import ExitStack

import numpy as np

for _p in ("/opt/pypackages", "/opt/trn_rl_repo"):
    if _p not in sys.path:
        sys.path.append(_p)

import concourse.bass as bass
import concourse.bacc as bacc
import concourse.mybir as mybir
from concourse.bass_utils import run_bass_kernel_spmd
from concourse.masks import make_identity
from concourse.tile import TileContext

F32 = mybir.dt.float32
BF16 = mybir.dt.bfloat16
I16 = mybir.dt.int16
ALU = mybir.AluOpType
ACTF = mybir.ActivationFunctionType


class Cfg:
    def __init__(self, H=100, W=100, C=256, PAD=8, n_cores=8, B=4):
        self.H, self.W, self.C, self.PAD = H, W, C, PAD
        self.B = B
        self.n_cores = n_cores
        self.halves = n_cores // B          # shards per sample (2)
        self.RS = H // self.halves          # rows per shard (50)
        self.WP = W + 2 * PAD               # padded row width (116)
        self.HP = H + 2 * PAD
        self.NROWS = self.HP * self.WP      # padded pixel rows (13456)
        self.NPX = self.RS * W              # real pixels per shard (5000)
        self.NPXP = -(-self.NPX // 2560) * 2560 if self.NPX > 2560 else 2560
        # pad pixel count to a multiple of 2560 when large, else one chunk
        if self.NPX <= 2560:
            self.CH = max(128, -(-self.NPX // 128) * 128)
            self.NPXP = self.CH
        else:
            self.CH = 2560
            self.NPXP = -(-self.NPX // self.CH) * self.CH
        self.NCHUNK = self.NPXP // self.CH
        self.NBLK = self.NPXP // 128        # px blocks (40)
        self.NM = self.NPXP // 16           # wrapped cols (320)
        self.CG = C // 128                  # channel groups (2)
        assert C % 128 == 0 and self.CG == 2, "kernel assumes C=256"
        assert self.NROWS < 32767


CFG = Cfg()

KH = (np.arange(9) // 3 - 1).astype(np.float32)
KW = (np.arange(9) % 3 - 1).astype(np.float32)
MAGIC = 12582912.0  # 1.5 * 2**23 : RNE-to-integer magic


def build_nc(cfg: Cfg, debug_dump=False):
    """Build the (SPMD, per-core identical) bass program."""
    nc = bacc.Bacc("TRN2", target_bir_lowering=False, debug=False,
                   num_swdge_queues=4)
    H, W, C, PAD = cfg.H, cfg.W, cfg.C, cfg.PAD
    WP, NROWS = cfg.WP, cfg.NROWS
    NPXP, CH, NBLK, NM = cfg.NPXP, cfg.CH, cfg.NBLK, cfg.NM
    NCHUNK = cfg.NCHUNK
    HW = H * W

    xin = nc.dram_tensor("xin", [C, H, W], F32, kind="ExternalInput")
    offP = nc.dram_tensor("offp", [128, NBLK, 18], F32, kind="ExternalInput")
    offW = nc.dram_tensor("offw", [128, NM, 18], F32, kind="ExternalInput")
    gyW = nc.dram_tensor("gyw", [128, NM], F32, kind="ExternalInput")
    gxW = nc.dram_tensor("gxw", [128, NM], F32, kind="ExternalInput")
    khw = nc.dram_tensor("khw", [128, 18], F32, kind="ExternalInput")  # [kh|kw]x9
    w0T = nc.dram_tensor("w0t", [128, 2], F32, kind="ExternalInput")
    b0s = nc.dram_tensor("b0s", [1, 1], F32, kind="ExternalInput")
    outd = nc.dram_tensor("out", [NPXP], F32, kind="ExternalOutput")

    if debug_dump:
        dbg_m = nc.dram_tensor("dbg_m", [128, 4, CH], BF16, kind="ExternalOutput")
        dbg_g0 = nc.dram_tensor("dbg_g0", [128, 4, CH], BF16, kind="ExternalOutput")
        dbg_g1 = nc.dram_tensor("dbg_g1", [128, 4, CH], BF16, kind="ExternalOutput")
        dbg_acc = nc.dram_tensor("dbg_acc", [128, 2, CH], BF16, kind="ExternalOutput")
        dbg_idx = nc.dram_tensor("dbg_idx", [128, 9, NM], I16, kind="ExternalOutput")
        dbg_ry = nc.dram_tensor("dbg_ry", [128, NM, 9], F32, kind="ExternalOutput")
        dbg_iyx = nc.dram_tensor("dbg_iyx", [128, NM, 18], F32, kind="ExternalOutput")
    xT = nc.dram_tensor("xT_pad", [NROWS, C], BF16, kind="Internal")
    xT_flat = bass.AP(tensor=xT.ap().tensor, offset=0, ap=[[1, NROWS * C]])
    # overlapping row-pair window view for the gather source
    xT_pairs = bass.AP(tensor=xT.ap().tensor, offset=0,
                       ap=[[C, NROWS - 1], [1, 2 * C]])

    with ExitStack() as ctx, TileContext(nc) as tc:
        # ---------------- constants -------------------------------------
        with tc.tile_pool(name="const", bufs=1) as pconst:
            ident = pconst.tile([128, 128], F32, name="ident")
            make_identity(nc, ident[:])
            w0sb = pconst.tile([128, 2], F32, name="w0sb")
            nc.sync.dma_start(out=w0sb[:], in_=w0T.ap())
            w0bf = pconst.tile([128, 2], BF16, name="w0bf")
            nc.vector.tensor_copy(out=w0bf[:], in_=w0sb[:])
            b0sb = pconst.tile([1, 1], F32, name="b0sb")
            nc.sync.dma_start(out=b0sb[:], in_=b0s.ap())
            khsb = pconst.tile([128, 9], F32, name="khsb")
            nc.sync.dma_start(out=khsb[:], in_=khw.ap()[:, 0:9])
            kwsb = pconst.tile([128, 9], F32, name="kwsb")
            nc.sync.dma_start(out=kwsb[:], in_=khw.ap()[:, 9:18])

            # ------------- phase A: build xT_pad ------------------------
            with tc.tile_pool(name="pa", bufs=2) as pa, \
                 tc.tile_pool(name="paz", bufs=1) as paz, \
                 tc.tile_pool(name="pap", bufs=6, space="PSUM") as pap:
                ztile = paz.tile([128, 2048], BF16, name="ztile")
                nc.gpsimd.memset(ztile[:], 0.0)
                # zero only the pad region (4 DMAs; disjoint from interior)
                band = PAD * WP * C  # top/bottom band elems
                assert band % 128 == 0 and band // 128 <= 2048
                xTt = xT.ap().tensor
                nc.sync.dma_start(
                    out=bass.AP(tensor=xTt, offset=0, ap=[[1, band]]),
                    in_=ztile[:, :band // 128])
                nc.sync.dma_start(
                    out=bass.AP(tensor=xTt, offset=(PAD + H) * WP * C,
                                ap=[[1, band]]),
                    in_=ztile[:, :band // 128])
                # left / right pad strips of each interior image row
                for off0 in (PAD * WP * C, (PAD * WP + PAD + W) * C):
                    nc.sync.dma_start(
                        out=bass.AP(tensor=xTt, offset=off0,
                                    ap=[[WP * C, H], [1, PAD * C]]),
                        in_=ztile[:H, :PAD * C])

                HB = 25 if H % 25 == 0 else 16  # h rows per staging batch
                assert H % HB == 0
                for g in range(cfg.CG):
                    xg = xin.ap().rearrange("(g p) h w -> g p (h w)", g=cfg.CG)[g]
                    for hb in range(H // HB):
                        xld = pa.tile([128, HB * W], F32, name="xld")
                        nc.sync.dma_start(
                            out=xld[:], in_=xg[:, hb * HB * W:(hb + 1) * HB * W])
                        stg = pa.tile([W, HB * 128], BF16, name="stg")
                        for j in range(HB):
                            pst = pap.tile([W, 128], F32, name="pst", space="PSUM")
                            nc.tensor.transpose(
                                out=pst[:], in_=xld[:, j * W:(j + 1) * W],
                                identity=ident[:])
                            if j % 2 == 0:
                                nc.scalar.activation(
                                    out=stg[:, j * 128:(j + 1) * 128], in_=pst[:],
                                    func=ACTF.Copy)
                            else:
                                nc.vector.tensor_copy(
                                    out=stg[:, j * 128:(j + 1) * 128], in_=pst[:])
                        # store: dst rows (h+PAD)*WP + (w+PAD), cols 128g..
                        h0 = hb * HB
                        dst = xT.ap().rearrange(
                            "(hp wp) c -> hp wp c", wp=WP)[
                                PAD + h0:PAD + h0 + HB, PAD:PAD + W,
                                g * 128:(g + 1) * 128]
                        dst = dst.rearrange("h w c -> w h c")
                        nc.sync.dma_start(out=dst, in_=stg[:].rearrange(
                            "w (h c) -> w h c", c=128))

            # ------------- phase B: weights + gather indices -------------
            idx_all = [None, None]
            maps = []
            with tc.tile_pool(name="pbs", bufs=1) as pbs:
                # px-part pipeline: corner weight maps [128, NBLK, 9] f32
                offPs = pbs.tile([128, NBLK, 18], F32, name="offPs")
                nc.sync.dma_start(out=offPs[:], in_=offP.ap())
                iyxP = pbs.tile([128, NBLK, 18], F32, name="iyxP")
                # floor(x) = rne(x - 0.5): (x - 0.5 + MAGIC) - MAGIC
                nc.vector.tensor_scalar(iyxP[:], offPs[:], 0.5, MAGIC,
                                        ALU.subtract, ALU.add)
                nc.vector.tensor_scalar(iyxP[:], iyxP[:], MAGIC, None,
                                        ALU.subtract)
                wyxP = pbs.tile([128, NBLK, 18], F32, name="wyxP")
                nc.vector.tensor_tensor(wyxP[:], offPs[:], iyxP[:], ALU.subtract)
                uyxP = pbs.tile([128, NBLK, 18], F32, name="uyxP")
                nc.vector.tensor_scalar(uyxP[:], wyxP[:], -1.0, 1.0,
                                        ALU.mult, ALU.add)
                wy = wyxP[:][:, :, 0::2]
                wx = wyxP[:][:, :, 1::2]
                uy = uyxP[:][:, :, 0::2]
                ux = uyxP[:][:, :, 1::2]
                for name, a_, b_ in (("m00", uy, ux), ("m01", uy, wx),
                                     ("m10", wy, ux), ("m11", wy, wx)):
                    m = pconst.tile([128, NBLK, 9], F32, name=name)
                    nc.vector.tensor_tensor(m[:], a_, b_, ALU.mult)
                    maps.append(m)

                # wrapped pipeline: gather indices [128, 9, NM] i16 per a
                offWs = pbs.tile([128, NM, 18], F32, name="offWs")
                nc.sync.dma_start(out=offWs[:], in_=offW.ap())
                gysb = pbs.tile([128, NM], F32, name="gysb")
                nc.sync.dma_start(out=gysb[:], in_=gyW.ap())
                gxsb = pbs.tile([128, NM], F32, name="gxsb")
                nc.sync.dma_start(out=gxsb[:], in_=gxW.ap())
                iyxW = pbs.tile([128, NM, 18], F32, name="iyxW")
                nc.vector.tensor_scalar(iyxW[:], offWs[:], 0.5, MAGIC,
                                        ALU.subtract, ALU.add)
                nc.vector.tensor_scalar(iyxW[:], iyxW[:], MAGIC, None,
                                        ALU.subtract)
                iyW = iyxW[:][:, :, 0::2]   # [128, NM, 9]
                ixW = iyxW[:][:, :, 1::2]
                # kh/kw broadcast over NM
                khb = khsb[:].rearrange("p (o n) -> p o n", o=1).to_broadcast(
                    [128, NM, 9])
                kwb = kwsb[:].rearrange("p (o n) -> p o n", o=1).to_broadcast(
                    [128, NM, 9])
                ry = pbs.tile([128, NM, 9], F32, name="ry")
                nc.vector.tensor_tensor(ry[:], iyW, khb, ALU.add)
                gyb = gysb[:].rearrange("p (m o) -> p m o", o=1).to_broadcast(
                    [128, NM, 9])
                nc.vector.tensor_tensor(ry[:], ry[:], gyb, ALU.add)
                cx = pbs.tile([128, NM, 9], F32, name="cx")
                nc.vector.tensor_tensor(cx[:], ixW, kwb, ALU.add)
                gxb = gxsb[:].rearrange("p (m o) -> p m o", o=1).to_broadcast(
                    [128, NM, 9])
                nc.vector.tensor_tensor(cx[:], cx[:], gxb, ALU.add)
                nc.vector.tensor_scalar(cx[:], cx[:], 0.0, float(WP - 2),
                                        ALU.max, ALU.min)
                r0 = pbs.tile([128, NM, 9], F32, name="r0")
                nc.vector.tensor_scalar(r0[:], ry[:], 0.0, float(WP - 2),
                                        ALU.max, ALU.min)
                r1 = pbs.tile([128, NM, 9], F32, name="r1")
                nc.vector.tensor_scalar(r1[:], ry[:], 1.0, 0.0, ALU.add, ALU.max)
                nc.vector.tensor_scalar(r1[:], r1[:], float(WP - 2), None, ALU.min)
                if debug_dump:
                    nc.sync.dma_start(out=dbg_ry.ap(), in_=r0[:])
                    nc.sync.dma_start(out=dbg_iyx.ap(), in_=iyxW[:])
                for a, rr in ((0, r0), (1, r1)):
                    idxf = pbs.tile([128, NM, 9], F32, name=f"idxf{a}")
                    nc.vector.tensor_scalar(idxf[:], rr[:], float(WP), None,
                                            ALU.mult)
                    nc.vector.tensor_tensor(idxf[:], idxf[:], cx[:], ALU.add)
                    idxi = pconst.tile([128, 9, NM], I16, name=f"idxi{a}")
                    nc.vector.tensor_copy(
                        out=idxi[:].rearrange("p t m -> p m t"), in_=idxf[:])
                    idx_all[a] = idxi
                if debug_dump:
                    nc.sync.dma_start(out=dbg_idx.ap(), in_=idx_all[0][:])

            tc.strict_bb_all_engine_barrier()

            # ------------- phase C: gather + blend + max ----------------
            accs = {}
            with tc.tile_pool(name="pg", bufs=2) as pg, \
                 tc.tile_pool(name="pm", bufs=2) as pm, \
                 tc.tile_pool(name="pacc", bufs=2) as pacc, \
                 tc.tile_pool(name="ps", bufs=2) as ps, \
                 tc.tile_pool(name="pmp", bufs=4, space="PSUM") as pmp, \
                 tc.tile_pool(name="pcv", bufs=2, space="PSUM") as pcv, \
                 tc.tile_pool(name="po", bufs=2) as po:
                for ch in range(NCHUNK):
                    acc0 = pacc.tile([128, CH], BF16, name="acc0")
                    acc1 = pacc.tile([128, CH], BF16, name="acc1")
                    accg = (acc0, acc1)
                    for t in range(9):
                        gt = []
                        SUB = 512  # dma_gather HW limit: < 1024 idxs/call
                        NS = CH // SUB
                        for a in range(2):
                            g = pg.tile([128, NS, 4, SUB], BF16, name=f"g{a}")
                            for s in range(NS):
                                m0 = ch * (CH // 16) + s * (SUB // 16)
                                nc.gpsimd.dma_gather(
                                    g[:, s], xT_pairs,
                                    idx_all[a][:][:, t, m0:m0 + SUB // 16],
                                    SUB, SUB, 2 * C, elem_step=C,
                                    transpose=True,
                                    queue_num=(2 * t + a) % 4)
                            gt.append(g)
                        # broadcast the 4 corner maps across partitions
                        msb = pm.tile([128, 4, CH], BF16, name="msb")
                        for blk in range(CH // 128):
                            bg = ch * (CH // 128) + blk
                            mp = pmp.tile([128, 512], F32, name="mp",
                                          space="PSUM")
                            for corner in range(4):
                                col = maps[corner][:].rearrange(
                                    "p b t -> p (b t)")[:, bg * 9 + t:
                                                        bg * 9 + t + 1]
                                nc.tensor.transpose(
                                    out=mp[:, corner * 128:(corner + 1) * 128],
                                    in_=col.to_broadcast([128, 128]),
                                    identity=ident[:])
                            nc.scalar.activation(
                                out=msb[:][:, :, blk * 128:(blk + 1) * 128],
                                in_=mp[:].rearrange("p (c n) -> p c n", n=128),
                                func=ACTF.Copy)
                        if debug_dump and ch == 0 and t == 0:
                            nc.sync.dma_start(out=dbg_m.ap(), in_=msb[:])
                            nc.sync.dma_start(out=dbg_g0.ap(), in_=gt[0][:].rearrange("p s c n -> p c (s n)"))
                            nc.sync.dma_start(out=dbg_g1.ap(), in_=gt[1][:].rearrange("p s c n -> p c (s n)"))
                        # blend + running max (per channel group)
                        def m3(c):
                            return msb[:, c].rearrange("p (s n) -> p s n", n=SUB)
                        for grp in range(2):
                            sA = ps.tile([128, CH], BF16, name="sA")
                            sB = ps.tile([128, CH], BF16, name="sB")
                            sA3 = sA[:].rearrange("p (s n) -> p s n", n=SUB)
                            sB3 = sB[:].rearrange("p (s n) -> p s n", n=SUB)
                            acc3 = accg[grp][:].rearrange(
                                "p (s n) -> p s n", n=SUB)
                            nc.vector.tensor_tensor(
                                sA3, m3(0), gt[0][:][:, :, grp], ALU.mult)
                            nc.vector.tensor_tensor(
                                sB3, m3(1), gt[0][:][:, :, 2 + grp], ALU.mult)
                            nc.vector.tensor_tensor(sA[:], sA[:], sB[:], ALU.add)
                            nc.vector.tensor_tensor(
                                sB3, m3(2), gt[1][:][:, :, grp], ALU.mult)
                            nc.vector.tensor_tensor(sA[:], sA[:], sB[:], ALU.add)
                            nc.vector.tensor_tensor(
                                sB3, m3(3), gt[1][:][:, :, 2 + grp], ALU.mult)
                            if t == 0:
                                nc.vector.tensor_tensor(
                                    accg[grp][:], sA[:], sB[:], ALU.add)
                            else:
                                nc.vector.tensor_tensor(
                                    sA[:], sA[:], sB[:], ALU.add)
                                nc.vector.tensor_tensor(
                                    accg[grp][:], accg[grp][:], sA[:], ALU.max)
                    if debug_dump and ch == 0:
                        nc.sync.dma_start(out=dbg_acc.ap()[:, 0], in_=acc0[:])
                        nc.sync.dma_start(out=dbg_acc.ap()[:, 1], in_=acc1[:])
                    # ----- conv + sigmoid + store for this chunk ---------
                    osb = po.tile([1, CH], F32, name="osb")
                    for seg in range(CH // 512):
                        pc = pcv.tile([1, 512], F32, name="pc", space="PSUM")
                        for grp in range(2):
                            nc.tensor.matmul(
                                pc[:], w0bf[:][:, grp:grp + 1],
                                accg[grp][:][:, seg * 512:(seg + 1) * 512],
                                start=(grp == 0), stop=(grp == 1))
                        nc.scalar.activation(
                            out=osb[:, seg * 512:(seg + 1) * 512], in_=pc[:],
                            func=ACTF.Sigmoid, bias=b0sb[:], scale=1.0)
                    nc.sync.dma_start(
                        out=outd.ap()[ch * CH:(ch + 1) * CH],
                        in_=osb[:])
    nc.compile()
    return nc


def host_prep(cfg: Cfg, x, offset):
    """Per-core input maps. Core = b * halves + half."""
    H, W, PAD = cfg.H, cfg.W, cfg.PAD
    in_maps = []
    kh18 = np.zeros((128, 18), np.float32)
    kh18[:, 0:9] = KH[None, :]
    kh18[:, 9:18] = KW[None, :]
    for core in range(cfg.n_cores):
        b = core // cfg.halves
        half = core % cfg.halves
        h0 = half * cfg.RS
        npx = cfg.NPXP
        hs = np.full(npx, h0, np.int64)
        ws = np.zeros(npx, np.int64)
        ii = np.arange(cfg.NPX)
        hs[:cfg.NPX] = h0 + ii // W
        ws[:cfg.NPX] = ii % W
        offb = offset[b][:, hs, ws].astype(np.float32)  # [18, npx]
        i = np.arange(npx)
        # px-part layout [128, NBLK, 18]
        offp = np.zeros((128, cfg.NBLK, 18), np.float32)
        offp[i % 128, i // 128, :] = offb.T
        # wrapped-replicated layout [128, NM, 18]
        offw = np.zeros((128, cfg.NM, 18), np.float32)
        gyw = np.zeros((128, cfg.NM), np.float32)
        gxw = np.zeros((128, cfg.NM), np.float32)
        for r in range(8):
            offw[i % 16 + 16 * r, i // 16, :] = offb.T
            gyw[i % 16 + 16 * r, i // 16] = hs + PAD
            gxw[i % 16 + 16 * r, i // 16] = ws + PAD
        in_maps.append({
            "xin": np.ascontiguousarray(x[b], np.float32),
            "offp": offp, "offw": offw, "gyw": gyw, "gxw": gxw,
            "khw": kh18,
        })
    return in_maps


_NC_CACHE = {}


def get_nc(cfg: Cfg):
    key = (cfg.H, cfg.W, cfg.C, cfg.n_cores)
    if key not in _NC_CACHE:
        _NC_CACHE[key] = build_nc(cfg)
    return _NC_CACHE[key]


def kernel(x, offset, w0, b0, trace=False):
    cfg = CFG
    x = np.asarray(x, np.float32)
    offset = np.asarray(offset, np.float32)
    w0 = np.asarray(w0, np.float32)
    b0 = np.asarray(b0, np.float32)
    nc = get_nc(cfg)
    in_maps = host_prep(cfg, x, offset)
    w0t = w0.reshape(2, 128).T.copy()
    for m in in_maps:
        m["w0t"] = w0t
        m["b0s"] = b0.reshape(1, 1)
    if trace:
        try:
            import antenv.axon_hooks  # noqa: F401
        except ImportError:
            trace = False
    res = run_bass_kernel_spmd(nc, in_maps, core_ids=list(range(cfg.n_cores)),
                               trace=trace)
    B, H, W = cfg.B, cfg.H, cfg.W
    out = np.zeros((B, 1, H, W), np.float32)
    for core in range(cfg.n_cores):
        b = core // cfg.halves
        half = core % cfg.halves
        h0 = half * cfg.RS
        o = res.results[core]["out"][:cfg.NPX].reshape(cfg.RS, W)
        out[b, 0, h0:h0 + cfg.RS] = o
    if trace:
        kernel.last_results = res
    return out



# revision 76
# speedup vs baseline: 2.0593x; 2.0593x over previous
"""Trainium2 Bass kernel for nn_DeformSpaceAttention (deformable 3x3 unfold +
per-channel max over taps + 1x1 conv + sigmoid).

Strategy (8 cores, data parallel over (batch, H-half)); pixel-partition
layout with host-side index/weight precomputation:
  - Host builds, per sample, a zero-padded channels-last "quad" copy
    xq[(y+8)*116 + (x+8)] = [x(y,x,:), x(y,x+1,:), x(y+1,x,:),
    x(y+1,x+1,:)] in bf16 (plus an fp8 copy), i16 gather row indices and
    f32 bilinear corner weights for every (tap, pixel). PAD=8 absorbs all
    out-of-bounds bilinear reads (zero-padding semantics).
  - Device, per 512-pixel unit and tap: one SWDGE dma_gather
    (transpose=False) fetches all 4 corners' channels of each pixel into
    a pixel-partition tile [128px, 4blk, 4*C]; one of the 9 taps gathers
    from the fp8 copy to shave DMA bytes.
  - Corner multiplies run on DVE as tensor_scalar ops (per-partition f32
    scalar weight, bf16 data -> 4x DVE mode).
  - The 4-corner sum runs on PE as identity-matmul PSUM accumulation.
  - ACT copies PSUM (f32) to SBUF bf16; DVE keeps a running max over the
    9 taps, software-pipelined one tap behind so DVE never stalls on the
    PE->ACT roundtrip; unit tails (final max, 1x1 conv via STT with f32
    accumulate, sigmoid, store) are deferred into the next unit's taps so
    Pool desc-gen and the DMA engines never starve.
"""

import sys

import numpy as np

for _p in ("/opt/pypackages", "/opt/trn_rl_repo"):
    if _p not in sys.path:
        sys.path.append(_p)

import ml_dtypes

import concourse.bass as bass
import concourse.bacc as bacc
import concourse.mybir as mybir
from concourse.bass_utils import run_bass_kernel_spmd
from concourse.masks import make_identity
from concourse.tile import TileContext

F32 = mybir.dt.float32
BF16 = mybir.dt.bfloat16
F8 = mybir.dt.float8e4
I16 = mybir.dt.int16
FP8_TAPS = (4,)   # taps gathered in fp8 (halves their DMA bytes)
ALU = mybir.AluOpType
ACTF = mybir.ActivationFunctionType

BF16NP = ml_dtypes.bfloat16


class Cfg:
    def __init__(self, H=100, W=100, C=256, PAD=8, n_cores=8, B=4):
        self.H, self.W, self.C, self.PAD = H, W, C, PAD
        self.B = B
        self.n_cores = n_cores
        self.halves = n_cores // B          # shards per sample (2)
        self.RS = H // self.halves          # rows per shard (50)
        self.WP = W + 2 * PAD               # padded row width (116)
        self.HP = H + 2 * PAD
        self.NROWS = self.HP * self.WP      # padded pixel rows (13456)
        self.NPX = self.RS * W              # real pixels per shard (5000)
        self.UPX = 512                      # pixels per unit (4 blocks)
        self.NU = -(-self.NPX // self.UPX)  # units (10)
        self.NPXP = self.NU * self.UPX      # padded pixels (5120)
        self.NBLK = self.NPXP // 128        # pixel blocks (40)
        assert C == 256 and self.NROWS < 32767


CFG = Cfg()

KH = (np.arange(9) // 3 - 1).astype(np.float32)
KW = (np.arange(9) % 3 - 1).astype(np.float32)


def build_nc(cfg: Cfg):
    """Build the (SPMD, per-core identical) bass program."""
    nc = bacc.Bacc("TRN2", target_bir_lowering=False, debug=False,
                   num_swdge_queues=4)
    C = cfg.C
    NROWS = cfg.NROWS
    NU, NBLK = cfg.NU, cfg.NBLK

    xt = nc.dram_tensor("xt", [NROWS, 4 * C], BF16, kind="ExternalInput")
    xt8 = nc.dram_tensor("xt8", [NROWS, 4 * C], F8, kind="ExternalInput")
    idxd = nc.dram_tensor("idxd", [128, 9, NU, 32], I16,
                          kind="ExternalInput")
    mard = nc.dram_tensor("mard", [128, NBLK, 9, 4], F32,
                          kind="ExternalInput")
    w0d = nc.dram_tensor("w0d", [128, 2], BF16, kind="ExternalInput")
    b0d = nc.dram_tensor("b0d", [128, 1], F32, kind="ExternalInput")
    outd = nc.dram_tensor("out", [cfg.NPXP], F32, kind="ExternalOutput")

    # quad rows: each row holds the 4 bilinear corners' channels
    xT_quad = bass.AP(tensor=xt.ap().tensor, offset=0,
                      ap=[[4 * C, NROWS], [1, 4 * C]])
    xT_quad8 = bass.AP(tensor=xt8.ap().tensor, offset=0,
                       ap=[[4 * C, NROWS], [1, 4 * C]])

    with TileContext(nc) as tc:
        with tc.tile_pool(name="const", bufs=1) as pconst:
            identf = pconst.tile([128, 128], F32, name="identf")
            make_identity(nc, identf[:])
            identb = pconst.tile([128, 128], BF16, name="identb")
            nc.vector.tensor_copy(out=identb[:], in_=identf[:])
            idx_sb = pconst.tile([128, 9, NU, 32], I16, name="idx_sb")
            nc.sync.dma_start(out=idx_sb[:][:, :, 0:1, :],
                              in_=idxd.ap()[:, :, 0:1, :])
            nc.sync.dma_start(out=idx_sb[:][:, :, 1:NU, :],
                              in_=idxd.ap()[:, :, 1:NU, :])
            mar_sb = pconst.tile([128, NBLK, 9, 4], F32, name="mar_sb")
            nc.sync.dma_start(out=mar_sb[:], in_=mard.ap())
            w0sb = pconst.tile([128, 2], BF16, name="w0sb")
            nc.sync.dma_start(out=w0sb[:], in_=w0d.ap())
            b0sb = pconst.tile([128, 1], F32, name="b0sb")
            nc.sync.dma_start(out=b0sb[:], in_=b0d.ap())
            osb = pconst.tile([1, cfg.NPXP], F32, name="osb")

            with tc.tile_pool(name="pg", bufs=8) as pg, \
                 tc.tile_pool(name="pg8", bufs=3) as pg8, \
                 tc.tile_pool(name="pgc", bufs=3) as pgc, \
                 tc.tile_pool(name="pp", bufs=5) as pp, \
                 tc.tile_pool(name="pps", bufs=6, space="PSUM") as pps, \
                 tc.tile_pool(name="ppt", bufs=1, space="PSUM") as ppt, \
                 tc.tile_pool(name="ppo", bufs=1, space="PSUM") as ppo, \
                 tc.tile_pool(name="pat", bufs=2) as pat, \
                 tc.tile_pool(name="psmp", bufs=7) as psmp, \
                 tc.tile_pool(name="pacc", bufs=3) as pacc:
                qctr = 0
                deferred = []   # closures finishing the previous unit

                def emit_copies(dstslices, pshalf):
                    # PSUM (f32) -> SBUF bf16, one copy per psum half
                    for h in range(2):
                        nc.scalar.activation(
                            out=dstslices[h],
                            in_=pshalf[h][:].rearrange(
                                "p (a c) -> p a c", c=256),
                            func=ACTF.Copy)

                for u in range(NU):
                    # last unit: only 392 of 512 pixels are real; gather
                    # fewer rows (stale tile tails are masked by zero weights)
                    ni = cfg.NPX - u * cfg.UPX
                    ni = 512 if ni >= 512 else ni
                    acc = pacc.tile([128, 4, 256], BF16, name="acc")
                    ps_hist = {}    # tap -> psum half-pair (copies lag 1 tap)
                    smp_hist = {}   # tap -> smp tile (max lags 2 taps)
                    gc_hist = {}    # fp8 tap -> ACT-upconverted bf16 tile
                    g8_hist = {}    # fp8 tap -> raw fp8 gather tile
                    idxcols = 0 - (-ni // 16)
                    for t in range(9):
                        # queue must equal global-call-index % 4 so each
                        # DMASW sem lane (index % 8) sees one queue only
                        if t in FP8_TAPS:
                            g = pg8.tile([128, 4, 1024], F8, name="g8")
                            src = xT_quad8
                        else:
                            g = pg.tile([128, 4, 1024], BF16, name="g")
                            src = xT_quad
                        nc.gpsimd.dma_gather(
                            g[:], src,
                            idx_sb[:][:, t, u, 0:idxcols],
                            ni, ni, 4 * C,
                            transpose=False,
                            queue_num=qctr % 4)
                        qctr += 1
                        # corner multiplies: per-pixel scalar weights (4x DVE)
                        P = pp.tile([128, 4, 4, 256], BF16, name="P")
                        for b4 in range(4):
                            blk = u * 4 + b4
                            for ci in range(4):
                                gsl = g[:][:, b4,
                                           ci * 256:ci * 256 + 256]
                                nc.vector.tensor_scalar(
                                    P[:][:, ci, b4], gsl,
                                    mar_sb[:][:, blk, t, ci:ci + 1],
                                    None, ALU.mult)
                        # 4-corner sum on PE via identity-matmul accumulation
                        pshalf = []
                        for h in range(2):
                            ps = pps.tile([128, 512], F32, name="ps",
                                          space="PSUM")
                            for ci in range(4):
                                nc.tensor.matmul(
                                    ps[:], identb[:],
                                    P[:][:, ci, 2 * h:2 * h + 2],
                                    start=(ci == 0), stop=(ci == 3))
                            pshalf.append(ps)
                        ps_hist[t] = pshalf
                        # copies for tap t (psum -> sbuf bf16)
                        if t == 0:
                            dsts = [acc[:][:, 0:2], acc[:][:, 2:4]]
                        else:
                            smp = psmp.tile([128, 4, 256], BF16,
                                            name="smp")
                            smp_hist[t] = smp
                            dsts = [smp[:][:, 0:2], smp[:][:, 2:4]]
                        emit_copies(dsts, ps_hist.pop(t))
                        # max for tap t-2 runs now (its ACT copy had two
                        # taps of slack, riding out the fp8 upconvert
                        # bursts) so DVE never stalls on PE->ACT roundtrips
                        tx = t - 2
                        if tx >= 1:
                            nc.vector.tensor_tensor(
                                acc[:], acc[:], smp_hist.pop(tx)[:],
                                ALU.max)
                        if t == 8:
                            # pull max(7) in-loop (one tap of slack is
                            # enough); shortens the unit's deferred chain
                            nc.vector.tensor_tensor(
                                acc[:], acc[:], smp_hist.pop(7)[:],
                                ALU.max)
                        # drain deferred tail ops of the previous unit at
                        # taps where their dep chains are fully resolved
                        if deferred and (t == 0 or t >= 4):
                            deferred.pop(0)()
                    # defer this unit's tail work into the next unit's taps:
                    # max(7), max(8), then the 1x1 conv on PE (transpose acc
                    # to channel partitions, matmul with w0), sigmoid, store
                    def _mk_final(acc=acc, smp8=smp_hist.pop(8), u=u):
                        pt = ppt.tile([128, 8, 128], BF16, name="pt",
                                      space="PSUM")
                        at = pat.tile([128, 8, 128], BF16, name="at")
                        po = ppo.tile([1, 512], F32, name="po", space="PSUM")

                        def mk_max(smp):
                            def f():
                                nc.vector.tensor_tensor(
                                    acc[:], acc[:], smp[:], ALU.max)
                            return f

                        def transp():
                            for b4 in range(4):
                                for gr in range(2):
                                    nc.tensor.transpose(
                                        out=pt[:][:, 2 * b4 + gr],
                                        in_=acc[:][:, b4,
                                                   128 * gr:128 * gr + 128],
                                        identity=identb[:])
                            nc.scalar.activation(out=at[:], in_=pt[:],
                                                 func=ACTF.Copy)

                        def convmm():
                            for b4 in range(4):
                                for gr in range(2):
                                    nc.tensor.matmul(
                                        po[:][:, 128 * b4:128 * b4 + 128],
                                        w0sb[:][:, gr:gr + 1],
                                        at[:][:, 2 * b4 + gr],
                                        start=(gr == 0), stop=(gr == 1))

                        def sigstore():
                            nc.scalar.activation(
                                out=osb[:][:, 512 * u:512 * u + 512],
                                in_=po[:],
                                func=ACTF.Sigmoid, bias=b0sb[:][0:1, :],
                                scale=1.0)
                            dst = bass.AP(tensor=outd.ap().tensor,
                                          offset=512 * u, ap=[[1, 512]])
                            nc.sync.dma_start(
                                out=dst,
                                in_=osb[:][:, 512 * u:512 * u + 512])
                        return [mk_max(smp8), transp, convmm, sigstore]
                    assert not deferred, deferred
                    deferred = _mk_final()
                for f in deferred:
                    f()
    nc.compile()
    return nc


def host_prep(cfg: Cfg, x, offset):
    """Per-core input maps. Core = b * halves + half."""
    H, W, C, PAD, WP = cfg.H, cfg.W, cfg.C, cfg.PAD, cfg.WP
    NPX, NPXP, NU, NBLK = cfg.NPX, cfg.NPXP, cfg.NU, cfg.NBLK
    in_maps = []
    xts = []
    for b in range(cfg.B):
        xtp = np.zeros((cfg.HP, WP, C), dtype=BF16NP)
        xtp[PAD:PAD + H, PAD:PAD + W, :] = \
            x[b].transpose(1, 2, 0).astype(BF16NP)
        xf = xtp.reshape(cfg.NROWS, C)
        # quad rows: [x(r), x(r+1), x(r+WP), x(r+WP+1)] per row r
        xq = np.zeros((cfg.NROWS, 4 * C), dtype=BF16NP)
        xq[:, 0:C] = xf
        xq[:-1, C:2 * C] = xf[1:]
        xq[:-WP, 2 * C:3 * C] = xf[WP:]
        xq[:-WP - 1, 3 * C:4 * C] = xf[WP + 1:]
        xts.append((xq, xq.astype(ml_dtypes.float8_e4m3)))
    for core in range(cfg.n_cores):
        b = core // cfg.halves
        half = core % cfg.halves
        h0 = half * cfg.RS
        px = np.arange(NPXP)
        valid = (px < NPX).astype(np.float32)
        pxc = np.minimum(px, NPX - 1)
        hs = h0 + pxc // W
        ws = pxc % W
        offb = offset[b]
        dy = offb[0::2, hs, ws].astype(np.float32)     # [9, NPXP]
        dx = offb[1::2, hs, ws].astype(np.float32)
        py = hs[None].astype(np.float32) + KH[:, None] + dy
        pxx = ws[None].astype(np.float32) + KW[:, None] + dx
        y0 = np.floor(py)
        x0 = np.floor(pxx)
        wy = py - y0
        wx = pxx - x0
        y0c = np.clip(y0, -PAD, H + PAD - 2).astype(np.int32)
        x0c = np.clip(x0, -PAD, W + PAD - 2).astype(np.int32)
        row0 = ((y0c + PAD) * WP + (x0c + PAD)).astype(np.int16)  # [9, NPXP]
        # wrap-16 index layout: position k of call (t,u) -> [k%16, k//16]
        idxw16 = row0.reshape(9, NU, 32, 16).transpose(3, 0, 1, 2)
        idxd = np.ascontiguousarray(np.tile(idxw16, (8, 1, 1, 1)))
        uy = 1.0 - wy
        ux = 1.0 - wx
        mall = np.stack([uy * ux, uy * wx, wy * ux, wy * wx], -1)
        mall *= valid[None, :, None]                    # [9, NPXP, 4]
        mard = np.ascontiguousarray(
            mall.reshape(9, NBLK, 128, 4).transpose(2, 1, 0, 3),
            dtype=np.float32)
        in_maps.append({
            "xt": xts[b][0],
            "xt8": xts[b][1],
            "idxd": idxd,
            "mard": mard,
        })
    return in_maps


_NC_CACHE = {}


def get_nc(cfg: Cfg):
    key = (cfg.H, cfg.W, cfg.C, cfg.n_cores)
    if key not in _NC_CACHE:
        _NC_CACHE[key] = build_nc(cfg)
    return _NC_CACHE[key]


def kernel(x, offset, w0, b0, trace=False):
    cfg = CFG
    x = np.asarray(x, np.float32)
    offset = np.asarray(offset, np.float32)
    w0 = np.asarray(w0, np.float32)
    b0 = np.asarray(b0, np.float32)
    nc = get_nc(cfg)
    in_maps = host_prep(cfg, x, offset)
    w0b = np.ascontiguousarray(w0.reshape(2, 128).T, np.float32
                               ).astype(BF16NP)
    b0r = np.full((128, 1), float(b0[0]), np.float32)
    for m in in_maps:
        m["w0d"] = w0b
        m["b0d"] = b0r
    if trace:
        try:
            import antenv.axon_hooks  # noqa: F401
        except ImportError:
            trace = False
    res = run_bass_kernel_spmd(nc, in_maps, core_ids=list(range(cfg.n_cores)),
                               trace=trace)
    B, H, W = cfg.B, cfg.H, cfg.W
    out = np.zeros((B, 1, H, W), np.float32)
    for core in range(cfg.n_cores):
        b = core // cfg.halves
        half = core % cfg.halves
        h0 = half * cfg.RS
        o = res.results[core]["out"][:cfg.NPX].reshape(cfg.RS, W)
        out[b, 0, h0:h0 + cfg.RS] = o
    if trace:
        kernel.last_results = res
    return out
